# revision 138
# baseline (speedup 1.0000x reference)
"""Trainium2 Bass kernel for nn_MultiHeadAttention (B=4, T=1024, D=1024, H=16, dk=64).

Sharding: 8 cores = 4 batches x 2 head-groups (8 heads / 512 features each).
Each core computes a partial output (its head-group's contribution through Wo);
host sums the two partials per batch (the "all-reduce after linear_out" done
host-side during unshard) and adds bo.

Per-core dataflow (all on one NeuronCore, Tile-scheduled):
  A) q/k/v projections as X^T-major fp32r matmuls -> (Tq partitions, F free),
     drain + per-head LayerNorm (bn_stats + even/odd aggregation on DVE,
     normalize split Pool/DVE -> bf16 qhat), PE-transpose 128x128 blocks
     into qlnT/klnT (F partitions, T free), gamma applied on the single
     strided ACT drain of each transpose group. The transpose block of
     tile t is DEFERRED until after tile t+3's matmuls (PE is in-order;
     this hides the ~3us LN-chain latency).
  B) per head pair: scoresT = klnT.T @ qlnT (two 512-col halves, one PSUM
     bank each), one merged exp per (tk,head) on ACT (PSUM [128,1024] ->
     SBUF bf16; ACT is the attention roofline at ~1.04us/exp), mask
     multiply (bf16 DVE), x_aug = [v|1].T @ attnT accumulated over tk ->
     64 rows x + 64 rows denominator. Epilogue (DVE recip, DMA partition
     shift, DVE multiply) is inlined at tk=7 per head so the PSUM frees
     overlap the remaining heads.
  C) out = x_all.T @ WoT, 1-bank PSUM chunks from the 4-buf pool (first
     pair from the idle score pool to dodge the j3 epilogue), drains
     alternating ACT/DVE, store DMAs on SP.
Scheduling: HWDGE descriptor gen is a single serial ~625ns/DMA resource --
first-needed loads (xt(k,0..1) halves + wk chunks) are hoisted ahead,
small constants go via SWDGE, later weights are issued just before their
phase. 22 dummy identity transposes warm the PE p-state during the
startup DMA wait (cost model: half clock until 3us continuously busy).

Why the odd bits: walrus here allows only ONE sync-wait per instruction
(_split_excess_waits patches the BIR); custom-DVE reciprocal_approx and
SBUF->SBUF DMA-transpose are broken in this toolchain (see memory notes).
TimelineSim estimate (the graded metric): 147744 ns; baseline was 196673.
"""

import os
import numpy as np
import ml_dtypes

T = 1024
D = 1024
F = 512      # features per core (8 heads x 64)
NH = 8       # heads per core
DK = 64
P = 128
EPS = 1e-5
BF16 = ml_dtypes.bfloat16

_CACHE = {}

# feature knobs (for bisecting compiler issues)
USE_DMA_TRANSPOSE = False  # SBUF->SBUF xbar transpose corrupts under concurrent DMA traffic
RECIP_MODE = "exact"  # "lnexp" (ACT), "exact" (DVE), "split" (both)
DEBUG = False  # add intermediate dumps as extra outputs
USE_GPSIMD_MASK = True
MASK_GPSIMD_MOD = 0   # tk % MOD == MOD-1 goes to gpsimd; 0 disables
SC_BUFS = 2
PS512_BUFS = 4
ATTN_BUFS = 9
PHASES = 3  # 1=A only, 2=A+B, 3=full
SWDGE_LOADS = True  # route v/mask/wo loads through gpsimd SWDGE queues
FAKE_XT = False  # replace xt DMA loads with memsets (sim experiment)
NORM_ENGINE = "gpsimd"  # "gpsimd" or "vector"
DRAIN_BUFS = 4
STAT_BUFS = 4
QHAT_BUFS = 3
INTERLEAVE_KQ = False
MASK_PER_C = True
PDRAIN_ENGINE = "any"
C_OUTER = False
PE_SHIFT = False
V_ORDER = "last"
V_INTERLEAVE = False
PST_POOL = "sc"
XIN_BUFS = 4
GB_ALT = False
PE_SHIFT_LAST = False
A_STOP = 4  # 1=proj+drain 2=+stats 3=+normalize 4=full A
C_OUTER2 = False  # c-outer pipeline: tried, loses to merged-exp attention (ACT overhead doubles)


def _split_excess_waits(bj):
    """Walrus allows at most 1 sync-wait per instruction (2 for
    EventSemaphore). Tile's sem assigner can emit more; spill the excess
    onto NoOp carriers inserted just before, on the same engine."""
    import json
    d = json.loads(bj)
    ctr = 0
    for fn in d["functions"]:
        for bb in fn["blocks"]:
            new = []
            for inst in bb["instructions"]:
                si = inst.get("sync_info") or {}
                ow = si.get("on_wait") or []
                op = inst.get("opcode", "")
                cap = 2 if op == "EventSemaphore" else 1
                if len(ow) > cap:
                    for w in ow[:-cap]:
                        ctr += 1
                        new.append({
                            "debug": inst.get("debug", 0),
                            "engine": inst["engine"],
                            "ins": [], "outs": [],
                            "name": f"W-{ctr}",
                            "opcode": "NoOp",
                            "sync_info": {"on_update": [], "on_wait": [w]},
                            "text_hint": "waitsplit",
                        })
                    si["on_wait"] = ow[-cap:]
                new.append(inst)
            bb["instructions"] = new
    return json.dumps(d).encode(), ctr


def _build(use_bq, use_bk, use_bv, ln_beta_zero=True):
    import concourse.bass as bass
    import concourse.tile as tile
    from concourse import mybir

    f32 = mybir.dt.float32
    f32r = mybir.dt.float32r
    bf16 = mybir.dt.bfloat16

    nc = bass.Bass()


    # ---- DRAM I/O ----
    xq_t = nc.dram_tensor("xq_t", (D, T), f32r, kind="ExternalInput").ap()
    xk_t = nc.dram_tensor("xk_t", (D, T), f32r, kind="ExternalInput").ap()
    xv_t = nc.dram_tensor("xv_t", (D, T), f32r, kind="ExternalInput").ap()
    wq_t = nc.dram_tensor("wq_t", (D, F), f32r, kind="ExternalInput").ap()
    wk_t = nc.dram_tensor("wk_t", (D, F), f32r, kind="ExternalInput").ap()
    wv_t = nc.dram_tensor("wv_t", (D, F), f32r, kind="ExternalInput").ap()
    wo_t = nc.dram_tensor("wo_t", (F, D), f32r, kind="ExternalInput").ap()
    mask_t = nc.dram_tensor("mask_t", (T, T), bf16, kind="ExternalInput").ap()
    # per-partition LN constants (128,) = per (head-pair-local feature)
    gq_d = nc.dram_tensor("gq", (P, 1), f32, kind="ExternalInput").ap()
    bq_d = nc.dram_tensor("bq_ln", (P, 1), f32, kind="ExternalInput").ap()
    gk_d = nc.dram_tensor("gk", (P, 1), f32, kind="ExternalInput").ap()
    bk_d = nc.dram_tensor("bk_ln", (P, 1), f32, kind="ExternalInput").ap()
    biases = {}
    for name, used in (("bq", use_bq), ("bk", use_bk), ("bv", use_bv)):
        if used:
            biases[name] = nc.dram_tensor(name, (F,), f32, kind="ExternalInput").ap()
    if PE_SHIFT or PE_SHIFT_LAST:
        identr_d = nc.dram_tensor("identr_d", (P, P), f32r, kind="ExternalInput").ap()
    out_p = nc.dram_tensor("out_p", (T, D), f32, kind="ExternalOutput").ap()
    dbg = {}
    if DEBUG:
        dbg["qlnT"] = nc.dram_tensor("dbg_qlnT", (P, 4, T), bf16, kind="ExternalOutput").ap()
        dbg["klnT"] = nc.dram_tensor("dbg_klnT", (P, 4, T), bf16, kind="ExternalOutput").ap()
        dbg["vaug"] = nc.dram_tensor("dbg_vaug", (P, 8, NH, P), bf16, kind="ExternalOutput").ap()
        dbg["xall"] = nc.dram_tensor("dbg_xall", (P, 4, T), f32r, kind="ExternalOutput").ap()
        dbg["qsb0"] = nc.dram_tensor("dbg_qsb0", (P, NH, DK), f32, kind="ExternalOutput").ap()
        dbg["at00"] = nc.dram_tensor("dbg_at00", (P, T), bf16, kind="ExternalOutput").ap()

    # DRAM views
    xviews = {
        "q": xq_t.rearrange("(dc p) t -> p dc t", p=P),
        "k": xk_t.rearrange("(dc p) t -> p dc t", p=P),
        "v": xv_t.rearrange("(dc p) t -> p dc t", p=P),
    }
    wviews = {
        "q": wq_t.rearrange("(dc p) f -> p dc f", p=P),
        "k": wk_t.rearrange("(dc p) f -> p dc f", p=P),
        "v": wv_t.rearrange("(dc p) f -> p dc f", p=P),
    }
    wo_view = wo_t.rearrange("(fc p) d -> p fc d", p=P)
    mask_view = mask_t.rearrange("(kc p) t -> p kc t", p=P)
    out_view = out_p.rearrange("(tc p) d -> p tc d", p=P)

    with tile.TileContext(nc) as tc:
        with (
            tc.tile_pool(name="const", bufs=1) as const,
            tc.tile_pool(name="xin", bufs=XIN_BUFS) as xin,
            tc.tile_pool(name="drain", bufs=DRAIN_BUFS) as drain,
            tc.tile_pool(name="stat", bufs=STAT_BUFS) as stat,
            tc.tile_pool(name="qhatp", bufs=5) as qhatp,
            tc.tile_pool(name="attnp", bufs=ATTN_BUFS) as attnp,
            tc.tile_pool(name="recipp", bufs=3) as recipp,
            tc.tile_pool(name="outp", bufs=7) as outp,
            tc.tile_pool(name="ps512", bufs=(3 if C_OUTER2 else PS512_BUFS),
                         space="PSUM") as ps512,
            tc.tile_pool(name="ps1024", bufs=SC_BUFS, space="PSUM") as ps1024,
            tc.tile_pool(name="pscore", bufs=3, space="PSUM") as pscore,
            tc.tile_pool(name="pacc", bufs=2, space="PSUM") as pacc,
        ):
            # ---- resident tiles ----
            w_sb = {
                pn: const.tile([P, 8, F], f32r, name=f"w_{pn}", tag=f"w_{pn}") for pn in ("q", "k", "v")
            }
            wo_sb = const.tile([P, 4, D], f32r, name="wo", tag="wo")
            qlnT = const.tile([P, 4, T], bf16, name="qlnT", tag="qlnT")
            klnT = const.tile([P, 4, T], bf16, name="klnT", tag="klnT")
            vaug = const.tile([P, 8, NH, P], bf16, name="vaug", tag="vaug")  # [p, tk, h, 128]
            mask_sb = const.tile([P, 8, T], bf16, name="mask", tag="mask")
            x_all = const.tile([P, 4, T], f32r, name="xall", tag="xall")
            eps_t = const.tile([P, 1], f32, name="eps", tag="eps")

            # HWDGE descriptor generation is a single serial resource
            # (~625ns/DMA): the first matmul needs only xt(k, t=0) and the
            # wk chunks, so those go first; everything else is pushed behind
            # them (small constants via SWDGE) or issued just before the
            # phase that consumes it.
            xt_first = xin.tile([P, 8, P], f32r, name="xt", tag="xt")
            nc.sync.dma_start(xt_first[:, 0:4, :], xviews["k"][:, 0:4, 0:P])
            for d in range(4):
                nc.sync.dma_start(w_sb["k"][:, d, :], wviews["k"][:, d, :])
            nc.sync.dma_start(xt_first[:, 4:8, :], xviews["k"][:, 4:8, 0:P])
            for d in range(4, 8):
                nc.sync.dma_start(w_sb["k"][:, d, :], wviews["k"][:, d, :])
            xt_second = xin.tile([P, 8, P], f32r, name="xt", tag="xt")
            nc.sync.dma_start(xt_second, xviews["k"][:, :, P:2 * P])

            from concourse.masks import make_identity
            ident = const.tile([P, P], bf16, name="ident", tag="ident")
            make_identity(nc, ident)
            gb_t = {}
            for nm, dr in (("gq", gq_d), ("bq", bq_d), ("gk", gk_d), ("bk", bk_d)):
                gb_t[nm] = const.tile([P, 1], f32, name=f"ln_{nm}", tag=f"ln_{nm}")
                nc.gpsimd.dma_start(gb_t[nm], dr)
            nc.vector.memset(eps_t, EPS)
            # warm the PE p-state during the startup DMA wait: the cost
            # model runs matmuls at half speed until the engine has been
            # continuously busy for 3us, so ~35 back-to-back dummy
            # transposes of the identity put the first real projection
            # matmuls at full clock
            warm_ps = ps512.tile([P, 2 * F], bf16, name="warm", tag="ps512")
            for _ in range(22):
                nc.tensor.transpose(warm_ps[:, 0:P], ident, ident)
            if PE_SHIFT or PE_SHIFT_LAST:
                identr = const.tile([P, P], f32r, name="identr", tag="identr")
                nc.gpsimd.dma_start(identr, identr_d)

            bias_bc = {}
            for name in biases:
                bias_bc[name] = const.tile([P, F], f32, name=f"bc_{name}", tag=f"bc_{name}")
                src = bass.AP(
                    tensor=biases[name].tensor,
                    offset=biases[name].offset,
                    ap=[[0, P], [1, F]],
                )
                nc.gpsimd.dma_start(out=bias_bc[name], in_=src)

            _dma2 = nc.gpsimd if SWDGE_LOADS else nc.sync
            # ones columns of v_aug: even h -> cols 64:128, odd h -> cols 0:64
            nc.gpsimd.memset(vaug[:, :, 0::2, DK:P], 1.0)
            nc.gpsimd.memset(vaug[:, :, 1::2, 0:DK], 1.0)

            def load_w(pn):
                for d in range(8):
                    nc.sync.dma_start(w_sb[pn][:, d, :], wviews[pn][:, d, :])

            def load_mask_wo():
                for tk in range(8):
                    nc.sync.dma_start(mask_sb[:, tk, :], mask_view[:, tk, :])
                for j in range(4):
                    nc.sync.dma_start(wo_sb[:, j, :], wo_view[:, j, :])

            ln_params = {"q": ("gq", "bq"), "k": ("gk", "bk")}

            # PE executes its queue in order, so a transpose that waits on
            # the LN chain of tile t would stall the projection matmuls of
            # t+1 behind it. Defer each tile's transposes by one iteration:
            # they are emitted right AFTER the next tile's matmuls, whose
            # execution hides the LN-chain latency.
            deferred = []

            def pop_deferred(keep=0):
                if len(deferred) > keep:
                    deferred.pop(0)()

            # ---- Phase A: projections + LN + transpose ----
            def ln_chain(pn, dstT, t, ps):
                """Post-matmul LN work for one projection tile: drain, stats,
                normalize (all non-PE), then append the deferred transpose
                block. Reused by proj_ln and the attention-phase fillers."""
                bias_name = "b" + pn
                sb = drain.tile([P, NH, DK], f32, name="qsb", tag="qsb")
                if bias_name in bias_bc:
                    nc.vector.tensor_add(
                        sb.rearrange("p h d -> p (h d)"), ps, bias_bc[bias_name])
                else:
                    deng = nc.vector if PDRAIN_ENGINE == "vector" else nc.any
                    deng.tensor_copy(
                        out=sb.rearrange("p h d -> p (h d)"), in_=ps)
                if A_STOP < 2:
                    return
                st = stat.tile([P, NH, 6], f32, name="st", tag="st")
                for h in range(NH):
                    nc.vector.bn_stats(out=st[:, h, :], in_=sb[:, h, :])
                # combine even/odd halves: mu=(me+mo)/2;
                # var=(32ve+32vo)/64 + ((me-mo)/2)^2
                me, mo = st[:, :, 1], st[:, :, 4]
                ve, vo = st[:, :, 2], st[:, :, 5]
                mu = stat.tile([P, NH], f32, name="mu", tag="mu")
                nc.vector.tensor_add(mu, me, mo)
                nc.vector.tensor_scalar_mul(mu, mu, 0.5)
                dm = stat.tile([P, NH], f32, name="dm", tag="dm")
                nc.vector.tensor_sub(dm, me, mo)
                nc.vector.tensor_scalar_mul(dm, dm, 0.5)
                nc.vector.tensor_mul(dm, dm, dm)  # ((me-mo)/2)^2
                sv = stat.tile([P, NH], f32, name="sv", tag="sv")
                nc.vector.tensor_add(sv, ve, vo)
                var = stat.tile([P, NH], f32, name="var", tag="var")
                # var = sv/64 + dm
                nc.vector.scalar_tensor_tensor(
                    out=var, in0=sv, scalar=1.0 / DK, in1=dm,
                    op0=mybir.AluOpType.mult,
                    op1=mybir.AluOpType.add)
                sd = stat.tile([P, NH], f32, name="sd", tag="sd")
                nc.scalar.activation(
                    out=sd, in_=var,
                    func=mybir.ActivationFunctionType.Sqrt,
                    bias=eps_t,
                )
                rs = stat.tile([P, NH], f32, name="rs", tag="rs")
                nc.vector.reciprocal(out=rs, in_=sd)
                if A_STOP < 3:
                    return
                qh = qhatp.tile([P, F], bf16, name="qh", tag="qh")
                for h in range(NH):
                    # split the per-head normalize between Pool and DVE:
                    # either alone saturates and stalls the transposes
                    norm_eng = nc.gpsimd if h % 2 == 0 else nc.vector
                    norm_eng.tensor_scalar(
                        out=qh[:, h * DK:(h + 1) * DK],
                        in0=sb[:, h, :],
                        scalar1=mu[:, h:h + 1],
                        scalar2=rs[:, h:h + 1],
                        op0=mybir.AluOpType.subtract,
                        op1=mybir.AluOpType.mult,
                    )
                if A_STOP < 4:
                    return
                g_nm, b_nm = ln_params[pn]

                def tr_block(qh=qh, t=t, dstT=dstT, g_nm=g_nm, b_nm=b_nm):
                    # all 4 transpose blocks land in one ps512-pool tile
                    # (1 bank) and drain with a single strided op,
                    # alternating ACT/DVE per tile
                    pst = ps512.tile([P, 2 * F], bf16, name="ps_bf", tag="ps512")
                    for j in range(4):
                        nc.tensor.transpose(
                            pst[:, j * P:(j + 1) * P], qh[:, j * P:(j + 1) * P], ident)
                    outap = dstT[:, :, t * P:(t + 1) * P]  # [P, 4, 128]
                    inap = pst[:, 0:F].rearrange("p (j c) -> p j c", c=P)
                    if ln_beta_zero:
                        nc.scalar.activation(
                            out=outap, in_=inap,
                            func=mybir.ActivationFunctionType.Copy,
                            scale=gb_t[g_nm],
                        )
                    else:
                        nc.vector.tensor_scalar(
                            out=outap, in0=inap,
                            scalar1=gb_t[g_nm], scalar2=gb_t[b_nm],
                            op0=mybir.AluOpType.mult, op1=mybir.AluOpType.add,
                        )

                deferred.append(tr_block)

            def proj_ln(pn, dstT, t_list=None, preloaded=None):
                for t in (t_list if t_list is not None else range(8)):
                    if preloaded and t in preloaded:
                        xt = preloaded.pop(t)
                    else:
                        xt = xin.tile([P, 8, P], f32r, name="xt", tag="xt")
                        nc.sync.dma_start(xt, xviews[pn][:, :, t * P:(t + 1) * P])
                    ps = ps512.tile([P, F], f32, name="ps512", tag="ps512")
                    for d in range(8):
                        nc.tensor.matmul(
                            ps, lhsT=xt[:, d, :], rhs=w_sb[pn][:, d, :],
                            start=(d == 0), stop=(d == 7),
                        )
                    pop_deferred(keep=3)
                    if A_STOP < 1:
                        continue
                    ln_chain(pn, dstT, t, ps)

            def v_proj(ts_list, use_sc=False, preloaded=None, prerun=False):
                for t in ts_list:
                    if preloaded and t in preloaded:
                        xt = preloaded.pop(t)
                    else:
                        xt = xin.tile([P, 8, P], f32r, name="xt", tag="xt")
                        nc.sync.dma_start(xt, xviews["v"][:, :, t * P:(t + 1) * P])
                    if use_sc:
                        # interleaved into attn j0: all ps512 bufs are held
                        # by the xps accumulators there
                        ps = ps1024.tile([P, T], f32, name="sc_v", tag="sc")[:, 0:F]
                    else:
                        ps = ps512.tile([P, F], f32, name="ps512", tag="ps512")
                    for d in range(8):
                        nc.tensor.matmul(
                            ps, lhsT=xt[:, d, :], rhs=w_sb["v"][:, d, :],
                            start=(d == 0), stop=(d == 7),
                        )
                    pop_deferred()
                    ps_h = ps.rearrange("p (hp two d) -> p hp two d", two=2, d=DK)
                    if "bv" in bias_bc:
                        vb = drain.tile([P, NH, DK], f32, name="vsb", tag="vsb")
                        nc.vector.tensor_add(
                            vb.rearrange("p h d -> p (h d)"), ps, bias_bc["bv"])
                        vb_h = vb.rearrange("p (hp two) d -> p hp two d", two=2)
                        nc.any.tensor_copy(out=vaug[:, t, 0::2, 0:DK], in_=vb_h[:, :, 0, :])
                        nc.any.tensor_copy(out=vaug[:, t, 1::2, DK:P], in_=vb_h[:, :, 1, :])
                    else:
                        nc.any.tensor_copy(out=vaug[:, t, 0::2, 0:DK], in_=ps_h[:, :, 0, :])
                        nc.any.tensor_copy(out=vaug[:, t, 1::2, DK:P], in_=ps_h[:, :, 1, :])
                    if prerun and t >= 4:
                        pre_attn(0, (t - 4) // 2, (t - 4) % 2)
                        if t >= 6:
                            pre_attn(0, 2 + (t - 6) // 2, (t - 6) % 2 + (t - 6) // 2 * 0)

            # pre-computed (scores+exp+mask) blocks: emitted inside the v
            # tail so the ACT exp stream (the attention bottleneck) starts
            # ~2 exps early; b_pair_full picks the at tiles up for its
            # accumulations
            preat = {}

            def pre_attn(j, tk, hh):
                h = 2 * j + hh
                rows = slice(hh * DK, (hh + 1) * DK)
                sp = ps1024.tile([P, T], f32, name="sc", tag="sc")
                lt = klnT[rows, j, tk * P:(tk + 1) * P]
                nc.tensor.matmul(sp[:, 0:F], lhsT=lt, rhs=qlnT[rows, j, 0:F],
                                 start=True, stop=True)
                nc.tensor.matmul(sp[:, F:T], lhsT=lt, rhs=qlnT[rows, j, F:T],
                                 start=True, stop=True)
                at = attnp.tile([P, T], bf16, name="attn_f", tag="attn")
                nc.scalar.activation(
                    out=at, in_=sp, func=mybir.ActivationFunctionType.Exp)
                # masks are applied in-phase by b_pair_full (their loads
                # land after v starts; DVE has fill-time slack there)
                preat[(j, tk, hh)] = at

            preat_half = {}

            def pre_attn_half(j, tk, hh):
                # c0-half score+exp only: needs just the first four q tiles
                # (transposed by proj_q's tail), so it can run while ACT
                # idles at the q->v boundary; b_pair_full completes the c1
                # half in-phase
                h = 2 * j + hh
                rows = slice(hh * DK, (hh + 1) * DK)
                sp = ps1024.tile([P, T], f32, name="sc", tag="sc")
                nc.tensor.matmul(
                    sp[:, 0:F], lhsT=klnT[rows, j, tk * P:(tk + 1) * P],
                    rhs=qlnT[rows, j, 0:F], start=True, stop=True)
                at = attnp.tile([P, T], bf16, name="attn_f", tag="attn")
                nc.scalar.activation(
                    out=at[:, 0:F], in_=sp[:, 0:F],
                    func=mybir.ActivationFunctionType.Exp)
                preat_half[(j, tk, hh)] = at

            proj_ln("k", klnT, preloaded={0: xt_first, 1: xt_second})
            # proj_k now outruns the DMA stream: hoist the first q input
            # tile ahead of the wq HWDGE gens (8 x 625ns) it would
            # otherwise queue behind
            xtq0 = xin.tile([P, 8, P], f32r, name="xt", tag="xt")
            nc.sync.dma_start(xtq0, xviews["q"][:, :, 0:P])
            load_w("q")
            if C_OUTER2:
                # c-outer pipeline: project q for the first tq half only;
                # the second half's tiles are projected one-per-head-pair
                # inside the attention c=0 sweep, whose ACT-bound phases
                # hide their PE cost. Likewise the first output-projection
                # groups hide inside the c=1 sweep.
                proj_ln("q", qlnT, [0, 1, 2, 3])
                load_w("v")
                v_proj(range(8))
                load_mask_wo()
            else:
                proj_ln("q", qlnT, preloaded={0: xtq0})
                pre_attn_half(0, 3, 0)
                pre_attn_half(0, 3, 1)
                pre_attn_half(0, 4, 0)
                pre_attn_half(0, 4, 1)
                xtv0 = xin.tile([P, 8, P], f32r, name="xt", tag="xt")
                nc.sync.dma_start(xtv0, xviews["v"][:, :, 0:P])
                load_w("v")
                if not V_INTERLEAVE:
                    v_proj(range(8), preloaded={0: xtv0}, prerun=True)
                load_mask_wo()
                if V_INTERLEAVE:
                    # flush the last deferred q-transposes before attention
                    # (normally drained by v_proj's iterations)
                    pop_deferred()
                    pop_deferred()
            # ---- Phase B: attention ----
            def b_pair(j, c):
                """Attention for head pair j over Tq half c (c-outer layout)."""
                xps = {}
                for hh in range(2):
                    xps[2 * j + hh] = ps512.tile([P, F], f32, name="ps512", tag="ps512")
                for tk in range(8):
                    for hh in range(2):
                        h = 2 * j + hh
                        rows = slice(hh * DK, (hh + 1) * DK)
                        sp = ps1024.tile([P, T], f32, name="sc", tag="sc")[:, 0:F]
                        nc.tensor.matmul(
                            sp, lhsT=klnT[rows, j, tk * P:(tk + 1) * P],
                            rhs=qlnT[rows, j, c * F:(c + 1) * F],
                            start=True, stop=True)
                        at = attnp.tile([P, F], bf16, name="attn", tag="attn")
                        nc.scalar.activation(
                            out=at, in_=sp, func=mybir.ActivationFunctionType.Exp)
                        nc.vector.tensor_mul(at, at, mask_sb[:, tk, c * F:(c + 1) * F])
                        if DEBUG and j == 0 and hh == 0 and tk == 0:
                            nc.sync.dma_start(out=dbg["at00"][:, c * F:(c + 1) * F], in_=at)
                        nc.tensor.matmul(
                            xps[h], lhsT=vaug[:, tk, h, :], rhs=at,
                            start=(tk == 0), stop=(tk == 7))
                for hh in range(2):
                    h = 2 * j + hh
                    xrows = slice(0, DK) if hh == 0 else slice(DK, P)
                    drows = slice(DK, P) if hh == 0 else slice(0, DK)
                    rc = recipp.tile([P, F], f32r, name="rc", tag="rc")
                    if RECIP_MODE == "lnexp" or (RECIP_MODE == "split" and c == 0):
                        lg = recipp.tile([P, F], f32r, name="lg", tag="lg")
                        nc.scalar.activation(
                            out=lg[drows], in_=xps[h][drows],
                            func=mybir.ActivationFunctionType.Ln)
                        nc.scalar.activation(
                            out=rc[drows], in_=lg[drows],
                            func=mybir.ActivationFunctionType.Exp, scale=-1.0)
                    else:
                        with nc.allow_low_precision(reason="f32r==f32 bits; recip of softmax denom"):
                            nc.vector.reciprocal(out=rc[drows], in_=xps[h][drows])
                    rsh = recipp.tile([P, F], f32r, name="rsh", tag="rsh")
                    nc.sync.dma_start(out=rsh[xrows], in_=rc[drows])
                    nc.vector.tensor_mul(
                        x_all[xrows, j, c * F:(c + 1) * F],
                        xps[h][xrows], rsh[xrows])

            def b_pair2(j, c, filler=None):
                """Attention for head pair j over tq half c. Holds only 2
                ps512 bufs (one [P,F] accumulator per head), leaving 2 for
                the filler work woven into the tk loop: PE is in-order, so
                the ACT-bound slack here is only usable by matmuls emitted
                INSIDE this loop (the filler generator yields after emitting
                one slot's worth of PE work)."""
                xps = {}
                for hh in range(2):
                    xps[2 * j + hh] = pacc.tile([P, F], f32, name="acc", tag="acc")
                for tk in range(8):
                    for hh in range(2):
                        h = 2 * j + hh
                        rows = slice(hh * DK, (hh + 1) * DK)
                        sp = pscore.tile([P, F], f32, name="scr", tag="scr")
                        nc.tensor.matmul(
                            sp, lhsT=klnT[rows, j, tk * P:(tk + 1) * P],
                            rhs=qlnT[rows, j, c * F:(c + 1) * F],
                            start=True, stop=True)
                        at = attnp.tile([P, F], bf16, name="attn", tag="attn")
                        nc.scalar.activation(
                            out=at, in_=sp, func=mybir.ActivationFunctionType.Exp)
                        nc.vector.tensor_mul(at, at, mask_sb[:, tk, c * F:(c + 1) * F])
                        nc.tensor.matmul(
                            xps[h], lhsT=vaug[:, tk, h, :], rhs=at,
                            start=(tk == 0), stop=(tk == 7))
                        if filler is not None:
                            next(filler, None)
                        if tk == 7:
                            xrows = slice(0, DK) if hh == 0 else slice(DK, P)
                            drows = slice(DK, P) if hh == 0 else slice(0, DK)
                            rc = recipp.tile([P, F], f32r, name="rc", tag="rc")
                            with nc.allow_low_precision(reason="f32r==f32 bits; recip of softmax denom"):
                                nc.vector.reciprocal(out=rc[drows], in_=xps[h][drows])
                            rsh = recipp.tile([P, F], f32r, name="rsh", tag="rsh")
                            nc.sync.dma_start(out=rsh[xrows], in_=rc[drows])
                            nc.vector.tensor_mul(
                                x_all[xrows, j, c * F:(c + 1) * F],
                                xps[h][xrows], rsh[xrows])

            def b_pair_full(j, with_v=False, pe_shift=False):
                xps = {}
                for hh in range(2):
                    h = 2 * j + hh
                    xps[h] = [ps512.tile([P, F], f32, name="ps512", tag="ps512")
                              for _ in range(2)]
                for tk in range(8):
                    if with_v:
                        v_proj([tk], use_sc=True)
                    for hh in range(2):
                        h = 2 * j + hh
                        rows = slice(hh * DK, (hh + 1) * DK)
                        if (j, tk, hh) in preat:
                            at = preat.pop((j, tk, hh))
                            for c in range(2):
                                nc.vector.tensor_mul(
                                    at[:, c * F:(c + 1) * F], at[:, c * F:(c + 1) * F],
                                    mask_sb[:, tk, c * F:(c + 1) * F])
                        elif (j, tk, hh) in preat_half:
                            at = preat_half.pop((j, tk, hh))
                            sp = ps1024.tile([P, T], f32, name="sc", tag="sc")
                            nc.tensor.matmul(
                                sp[:, F:T], lhsT=klnT[rows, j, tk * P:(tk + 1) * P],
                                rhs=qlnT[rows, j, F:T], start=True, stop=True)
                            nc.scalar.activation(
                                out=at[:, F:T], in_=sp[:, F:T],
                                func=mybir.ActivationFunctionType.Exp)
                            for c in range(2):
                                nc.vector.tensor_mul(
                                    at[:, c * F:(c + 1) * F], at[:, c * F:(c + 1) * F],
                                    mask_sb[:, tk, c * F:(c + 1) * F])
                        else:
                            sp = ps1024.tile([P, T], f32, name="sc", tag="sc")
                            lt = klnT[rows, j, tk * P:(tk + 1) * P]
                            # matmul output must stay within one PSUM bank
                            # (512 fp32), so scores go in two halves
                            nc.tensor.matmul(sp[:, 0:F], lhsT=lt, rhs=qlnT[rows, j, 0:F],
                                             start=True, stop=True)
                            nc.tensor.matmul(sp[:, F:T], lhsT=lt, rhs=qlnT[rows, j, F:T],
                                             start=True, stop=True)
                            at = attnp.tile([P, T], bf16, name="attn_f", tag="attn")
                            nc.scalar.activation(
                                out=at, in_=sp, func=mybir.ActivationFunctionType.Exp)
                            for c in range(2):
                                nc.vector.tensor_mul(
                                    at[:, c * F:(c + 1) * F], at[:, c * F:(c + 1) * F],
                                    mask_sb[:, tk, c * F:(c + 1) * F])
                        for c in range(2):
                            nc.tensor.matmul(
                                xps[h][c], lhsT=vaug[:, tk, h, :],
                                rhs=at[:, c * F:(c + 1) * F],
                                start=(tk == 0), stop=(tk == 7))
                        if DEBUG and j == 0 and hh == 0 and tk == 0:
                            nc.sync.dma_start(out=dbg["at00"], in_=at)
                        if tk == 7:
                            # inline the epilogue right after this head's last
                            # accumulation: its recip->shift->mul chain (and
                            # the xps buffer frees the next phase waits on)
                            # overlaps the remaining heads' work instead of
                            # trailing them
                            xrows = slice(0, DK) if hh == 0 else slice(DK, P)
                            drows = slice(DK, P) if hh == 0 else slice(0, DK)
                            for c in range(2):
                                rc = recipp.tile([P, F], f32r, name="rc", tag="rc")
                                with nc.allow_low_precision(reason="f32r==f32 bits; recip of softmax denom"):
                                    nc.vector.reciprocal(out=rc[drows], in_=xps[h][c][drows])
                                rsh = recipp.tile([P, F], f32r, name="rsh", tag="rsh")
                                nc.sync.dma_start(out=rsh[xrows], in_=rc[drows])
                                nc.vector.tensor_mul(
                                    x_all[xrows, j, c * F:(c + 1) * F],
                                    xps[h][c][xrows], rsh[xrows])

            def c_group(t, use_sc=False):
                # ps512 pool (1 bank/tile, 4 bufs) so the drain+DMA of one
                # output chunk overlaps the matmuls of the next three. The
                # first chunk pair can use the sc pool instead: right after
                # attn j=3 the ps512 bufs are still held by its xps
                # accumulators, while the sc (score) bufs free as soon as
                # the last exp drains — so the output projection starts
                # ~2us earlier.
                for n in range(2):
                    if use_sc:
                        ps = ps1024.tile([P, T], f32, name="sc_c", tag="sc")[:, 0:F]
                    else:
                        ps = ps512.tile([P, F], f32, name="ps512", tag="ps512")
                    for jj in range(4):
                        nc.tensor.matmul(
                            ps, lhsT=x_all[:, jj, t * P:(t + 1) * P],
                            rhs=wo_sb[:, jj, n * F:(n + 1) * F],
                            start=(jj == 0), stop=(jj == 3),
                        )
                    ob = outp.tile([P, F], f32, name="ob", tag="ob")
                    if (2 * t + n) % 2 == 0:
                        nc.scalar.activation(
                            out=ob, in_=ps,
                            func=mybir.ActivationFunctionType.Copy)
                    else:
                        nc.vector.tensor_copy(out=ob, in_=ps)
                    nc.sync.dma_start(out=out_view[:, t, n * F:(n + 1) * F], in_=ob)

            def make_q_filler(j):
                # projects q tile t=4+j, one matmul per attention slot; the
                # xt load is issued eagerly so it lands before slot 0
                t = 4 + j
                xt = xin.tile([P, 8, P], f32r, name="xt", tag="xt")
                nc.sync.dma_start(xt, xviews["q"][:, :, t * P:(t + 1) * P])

                def gen():
                    ps = ps512.tile([P, F], f32, name="ps512", tag="ps512")
                    for d in range(8):
                        nc.tensor.matmul(
                            ps, lhsT=xt[:, d, :], rhs=w_sb["q"][:, d, :],
                            start=(d == 0), stop=(d == 7),
                        )
                        yield
                    ln_chain("q", qlnT, t, ps)
                    yield
                    pop_deferred(keep=1)
                    yield
                return gen()

            def make_c_filler(j):
                def gen():
                    if j == 0:
                        pop_deferred()  # last deferred q transpose
                        yield
                    for n in range(2):
                        ps = ps512.tile([P, F], f32, name="ps512", tag="ps512")
                        for jj in range(4):
                            nc.tensor.matmul(
                                ps, lhsT=x_all[:, jj, j * P:(j + 1) * P],
                                rhs=wo_sb[:, jj, n * F:(n + 1) * F],
                                start=(jj == 0), stop=(jj == 3),
                            )
                            yield
                        ob = outp.tile([P, F], f32, name="ob", tag="ob")
                        if n == 0:
                            nc.scalar.activation(
                                out=ob, in_=ps,
                                func=mybir.ActivationFunctionType.Copy)
                        else:
                            nc.vector.tensor_copy(out=ob, in_=ps)
                        nc.sync.dma_start(
                            out=out_view[:, j, n * F:(n + 1) * F], in_=ob)
                        yield
                return gen()

            if C_OUTER2:
                for j in range(4):
                    b_pair2(j, 0, filler=make_q_filler(j))
                for j in range(4):
                    b_pair2(j, 1, filler=make_c_filler(j))
                for t in range(4, 8):
                    c_group(t)
            elif PHASES >= 2:
                for j in range(4):
                    b_pair_full(j, with_v=(V_INTERLEAVE and j == 0),
                                pe_shift=(PE_SHIFT_LAST and j == 3))
                if PHASES >= 3:
                    for t in range(4):
                        c_group(t, use_sc=(t == 0))
                    for t in range(4, 8):
                        c_group(t)

    return nc


def _get_nc(flags):
    if len(flags) == 3:
        flags = (*flags, True)
    key = (flags, USE_DMA_TRANSPOSE, RECIP_MODE, USE_GPSIMD_MASK, DEBUG,
           MASK_GPSIMD_MOD, SC_BUFS, PS512_BUFS, ATTN_BUFS, PHASES, SWDGE_LOADS, A_STOP, FAKE_XT, NORM_ENGINE, DRAIN_BUFS, STAT_BUFS, QHAT_BUFS, INTERLEAVE_KQ, MASK_PER_C, PDRAIN_ENGINE, C_OUTER, PE_SHIFT, V_ORDER, V_INTERLEAVE, PST_POOL, XIN_BUFS, GB_ALT, PE_SHIFT_LAST, C_OUTER2)
    if key not in _CACHE:
        nc = _build(*flags)
        patched, _n = _split_excess_waits(nc.to_json_bytes())
        nc.to_json_bytes = lambda: patched
        _CACHE[key] = nc
    return _CACHE[key]


def kernel(query, key, value, mask, Wq, bq, Wk, bk, Wv, bv, Wo, bo,
           q_gamma, q_beta, k_gamma, k_beta, _trace=False):
    from concourse.bass_utils import run_bass_kernel_spmd

    query = np.ascontiguousarray(np.asarray(query, np.float32))
    key = np.ascontiguousarray(np.asarray(key, np.float32))
    value = np.ascontiguousarray(np.asarray(value, np.float32))
    mask = np.asarray(mask)
    Wq, Wk, Wv, Wo = (np.asarray(w, np.float32) for w in (Wq, Wk, Wv, Wo))
    bq, bk, bv, bo = (np.asarray(b, np.float32) for b in (bq, bk, bv, bo))
    q_gamma, q_beta, k_gamma, k_beta = (
        np.asarray(g, np.float32) for g in (q_gamma, q_beta, k_gamma, k_beta))

    B = query.shape[0]
    use_bq, use_bk, use_bv = (bool(np.any(b)) for b in (bq, bk, bv))
    ln_beta_zero = not (np.any(q_beta) or np.any(k_beta))
    nc = _get_nc((use_bq, use_bk, use_bv, ln_beta_zero))

    # host-side shard prep
    xqT = [np.ascontiguousarray(query[b].T) for b in range(B)]
    xkT = [np.ascontiguousarray(key[b].T) for b in range(B)]
    xvT = [np.ascontiguousarray(value[b].T) for b in range(B)]
    maskT = [np.ascontiguousarray((~mask[b]).T.astype(BF16)) for b in range(B)]
    gq8 = np.ascontiguousarray((np.tile(q_gamma, 2) / 8.0).reshape(P, 1))
    bq8 = np.ascontiguousarray((np.tile(q_beta, 2) / 8.0).reshape(P, 1))
    gk2 = np.ascontiguousarray(np.tile(k_gamma, 2).reshape(P, 1))
    bk2 = np.ascontiguousarray(np.tile(k_beta, 2).reshape(P, 1))

    in_maps = []
    for core in range(8):
        b, g = core // 2, core % 2
        sl = slice(g * F, (g + 1) * F)
        im = {
            "xq_t": xqT[b], "xk_t": xkT[b], "xv_t": xvT[b],
            **({"identr_d": np.ascontiguousarray(np.eye(P, dtype=np.float32))}
               if (PE_SHIFT or PE_SHIFT_LAST) else {}),
            "wq_t": np.ascontiguousarray(Wq[sl].T),
            "wk_t": np.ascontiguousarray(Wk[sl].T),
            "wv_t": np.ascontiguousarray(Wv[sl].T),
            "wo_t": np.ascontiguousarray(Wo[:, sl].T),
            "mask_t": maskT[b],
            "gq": gq8, "bq_ln": bq8, "gk": gk2, "bk_ln": bk2,
        }
        if use_bq:
            im["bq"] = np.ascontiguousarray(bq[sl])
        if use_bk:
            im["bk"] = np.ascontiguousarray(bk[sl])
        if use_bv:
            im["bv"] = np.ascontiguousarray(bv[sl])
        in_maps.append(im)

    res = run_bass_kernel_spmd(nc, in_maps, core_ids=list(range(8)), trace=_trace)
    out = np.zeros((B, T, D), np.float32)
    for b in range(B):
        out[b] = res.results[2 * b]["out_p"] + res.results[2 * b + 1]["out_p"] + bo
    if _trace:
        kernel._last_results = res
    return out

